# revision 13
# baseline (speedup 1.0000x reference)
"""Trainium2 Bass kernel for the (misordered-scale) MultiHeadAttention problem.

Problem (per batch b of 8, one NeuronCore each):
  qk = x @ Wqk.T + bqk            # [2048, 512], channel c = 2*(h*64+e) + {0:q, 1:k}
  v  = x @ Wv.T  + bv             # [2048, 256], channel c = h*64+e
  S_h = q_h @ k_h.T               # [2048, 2048] per head (e=64)
  attn = softmax(S, -1) / 16
  out_h = attn_h @ v_h            # [2048, 64]
  out = concat_h(out_h) @ Wo.T + bo   # [2048, 1024]

Strategy: data-parallel over batch across 8 cores (no collectives).

Schedule (single dense PE stream, cost-model-guided):
  - x / Wq / Wk / Wv are consumed as float32r straight from HBM via 4-byte
    DMA-transposes (two 64-partition half-d calls each) -- no fp16 cast pass,
    so the first projection chain starts ~4us in.  fp32r streams at the same
    1 cycle/row as fp16 for >=256-wide outputs.
  - Projections run as 8-matmul accumulation chains double-buffered across
    two PSUM tags; q(ib=0) + all k + all v run up front (in x-arrival
    order), q(ib=1..3) chains are issued inside the attention phase as PE
    filler.
  - Attention is software-pipelined per (ib, head-pair): S(jb+1) is issued
    between exp(jb) (ACT, fused bias -8) and AV(jb); out-projection tiles and
    q-chains fill the PE gap left by the slower ACT exp stream.
  - The AV stationary operand [v_h | 16.0] makes PSUM row 64 the
    softmax denominator times 16 (folding the 1/16 mis-scale for free).
  - Normalization: one K=4 rank-4 PE matmul per (ib, pair) broadcasts both
    heads' 1/(16*denom) (bf16 hi/lo split for ~2^-16 accuracy) along
    partitions.
  - Wo stays fp16 (cast + transpose DMA); out tiles are evacuated with the
    bo broadcast on DVE and stored straight from SBUF.
"""

import numpy as np
from contextlib import ExitStack

import concourse.bass as bass
import concourse.mybir as mybir
import concourse.tile as tile
from concourse import bacc
from concourse import bass_utils

FP32 = mybir.dt.float32
FP32R = mybir.dt.float32r
BF16 = mybir.dt.bfloat16
FP16 = mybir.dt.float16
AF = mybir.ActivationFunctionType
ALU = mybir.AluOpType

B = 8
N = 2048          # tokens per batch
D = 1024          # model dim
H = 4             # heads
E = 64            # per-head dim after the einops split
HD = 256          # H*E (v channels / Wo contraction dim)
NCORES = 8

DC = D // 128      # 8 d-chunks of 128
NIB = N // 512     # 4 i-blocks of 512
NJB = N // 128     # 16 j-blocks of 128
# exp(S - 8): headroom offset for the exp path (max logit ~51 -> e^43 fits
# bf16 comfortably); the offset cancels exactly in the softmax normalization.
EXP_BIAS = -8.0
# AV ones-column value: makes PSUM row 64 equal 16*sum(exp), so its
# reciprocal is directly the softmax/16 normalization factor.
DEN_SCALE = 16.0

N_WARM = 14        # PE warmup matmuls (keep PE busy until first chain is fed)


def _build_kernel(nc: bass.Bass, tc: tile.TileContext, out_ap, x, wqk, bqk, wv, bv, wo, bo):
    ctx = ExitStack()
    with ctx:
        consts = ctx.enter_context(tc.tile_pool(name="consts", bufs=1))
        dram = ctx.enter_context(tc.tile_pool(name="dram", bufs=1, space="DRAM"))
        exps_pool = ctx.enter_context(tc.tile_pool(name="exps", bufs=4))
        osb_pool = ctx.enter_context(tc.tile_pool(name="osb", bufs=4))
        norm_pool = ctx.enter_context(tc.tile_pool(name="norm", bufs=2))
        ps = ctx.enter_context(tc.tile_pool(name="ps", bufs=3, space="PSUM"))

        # ---------------- persistent SBUF tensors ----------------
        xt = consts.tile([128, DC, N], FP16)          # x^T: xt[p, dc, t] = x[t, p*8+dc]
        wqT = consts.tile([128, DC, HD], FP16)        # Wq^T: [d, c]
        wkT = consts.tile([128, DC, HD], FP16)
        wvT = consts.tile([128, DC, HD], FP16)
        woT = consts.tile([128, 2, D], FP16)          # Wo^T: [c, do]
        qT = consts.tile([128, 2, N], FP16)           # q^T: [c, i]; c = cc*128 + p
        kT = consts.tile([128, 2, N], FP16)
        yT = consts.tile([128, 2, N], FP16)           # concat-head attn out, feature-major
        vh = consts.tile([128, NJB, H, 66], BF16)     # [j, jb, h, 0:64]=v_h, [...,64]=16.0
        bq_sb = consts.tile([128, 2, 1], FP32)        # q bias per partition (c)
        bk_sb = consts.tile([128, 2, 1], FP32)
        bv_row = consts.tile([1, HD], FP32)           # v bias as K=1 matmul rhs
        bo_row = consts.tile([1, D], FP32)
        ones32 = consts.tile([1, 128], FP32)
        wdum = consts.tile([1, 128], FP16)            # warmup stationary
        rdum = consts.tile([1, 512], FP16)            # warmup moving
        # Rank-4 norm broadcast lhsT.  Engine writes must start at a partition
        # quadrant (0/32/64/96), so the 4 logical rows (rhi0, rhi1, rlo0,
        # rlo1) live at partitions 0/32/64/96 of a 97-partition tile whose
        # other rows are hard zeros; the matmul contracts over all 97 rows at
        # the same cost (PE time only depends on the 512-wide output).
        nlhsT = consts.tile([97, 128], BF16)
        expb = consts.tile([128, 1], FP32)
        vbc = consts.tile([128, HD], FP32)            # bv broadcast over tokens
        obc = consts.tile([128, 2, 512], FP32)        # bo broadcast over tokens
        nc.vector.memset(ones32[:], 1.0)
        nc.vector.memset(wdum[:], 0.0)
        nc.vector.memset(rdum[:], 0.0)
        nc.vector.memset(expb[:], EXP_BIAS)
        nc.vector.memset(vh[:, :, :, 64:66], DEN_SCALE)
        nc.vector.memset(nlhsT[:], 0.0)
        nc.vector.memset(nlhsT[0:1, 0:64], 1.0)       # rhi h0 -> out rows 0:64
        nc.vector.memset(nlhsT[32:33, 64:128], 1.0)   # rhi h1 -> out rows 64:128
        nc.vector.memset(nlhsT[64:65, 0:64], 1.0)     # rlo h0
        nc.vector.memset(nlhsT[96:97, 64:128], 1.0)   # rlo h1
        # The rec / rhs4 norm tiles are written only at partition-quadrant
        # rows; zero both rotating buffers once so the filler rows contract
        # as exact zeros (never NaN garbage).
        for _ in range(2):
            rz = norm_pool.tile([33, 512], FP32, tag="rec")
            nc.vector.memset(rz[:], 0.0)
            hz = norm_pool.tile([97, 512], BF16, tag="rhs4")
            nc.vector.memset(hz[:], 0.0)

        # ---------------- DRAM staging (fp16 casts) ----------------
        # One DRAM tile per transfer: the tile scheduler tracks DRAM tiles at
        # whole-tile granularity, so a shared staging buffer would serialize
        # every cast against every transpose.
        xb_stage = [dram.tile([512, D], FP16, name=f"xst{i}") for i in range(4)]
        wq_bf = dram.tile([HD, D], FP16)
        wk_bf = dram.tile([HD, D], FP16)
        wv_bf = dram.tile([HD, D], FP16)
        wo_bf = dram.tile([D, HD], FP16)

        # De-interleave Wqk rows: q rows are 2c, k rows are 2c+1.
        wqk_r = wqk.rearrange("(c s) d -> s c d", s=2)
        bqk_r = bqk.rearrange("(c s) -> s c", s=2)

        # bias loads ride the ACT HWDGE ring (cheap, off the SP transpose ring)
        nc.scalar.dma_start(bv_row[0:1, :], bv[:])
        nc.scalar.dma_start(bo_row[0:1, :], bo[:])
        for cb in range(2):
            nc.scalar.dma_start(bq_sb[:, cb, :], bqk_r[0, cb * 128:(cb + 1) * 128])
            nc.scalar.dma_start(bk_sb[:, cb, :], bqk_r[1, cb * 128:(cb + 1) * 128])

        # ---------------- casts + transposes, supply order ----------------
        # The d (contraction) axis uses a permuted internal layout: SBUF chunk
        # dc at partition p holds global d = p*8 + dc (consistent between xt
        # and all W^T tiles; d is purely internal), so one full-width
        # transpose-DMA fills all 8 chunks via a 3D output AP.  Casts ride the
        # Pool SWDGE ring; x transposes the SP HWDGE ring and weight
        # transposes the ACT HWDGE ring (two rings keep two transfers in
        # flight while the casts flood the shared DMA device); the device
        # itself serves transfers roughly in arrival order, so the emission
        # order below is the supply schedule: x block 0 (split for an early
        # start) and the q/k weights first, wv before x block 1, then Wo.
        def x_sup(tb, t0, t1):
            st = xb_stage[tb]
            b0 = tb * 512
            nc.gpsimd.dma_start(st[t0 - b0:t1 - b0, :], x[t0:t1, :])
            nc.sync.dma_start(xt[:, :, t0:t1], st[t0 - b0:t1 - b0, :],
                              transpose=True)

        def w_sup(w_stage, dstT, src):
            nc.gpsimd.dma_start(w_stage[:], src)
            nc.sync.dma_start(dstT[:], w_stage[:], transpose=True)

        x_sup(0, 0, 256)
        w_sup(wq_bf, wqT, wqk_r[0])
        x_sup(0, 256, 512)
        w_sup(wk_bf, wkT, wqk_r[1])
        w_sup(wv_bf, wvT, wv[:])
        x_sup(1, 512, 1024)
        x_sup(2, 1024, 1536)
        x_sup(3, 1536, 2048)
        nc.gpsimd.dma_start(wo_bf[:], wo[:])
        for g in range(2):
            # Wo^T must match yT's c-layout (c = cc*128 + p): per-chunk.
            cs = slice(g * 128, (g + 1) * 128)
            nc.sync.dma_start(woT[:, g, :], wo_bf[:, cs], transpose=True)

        # ---------------- PE warmup + bias broadcasts ----------------
        # Keep the PE stream dense from ~1us so the p-state ramp is spent on
        # throwaway work, and the first real chains run at full clock.
        pb = ps.tile([128, 512], FP32, tag="sp")
        nc.tensor.matmul(pb[:, 0:HD], lhsT=ones32[:], rhs=bv_row[:],
                         start=True, stop=True)
        nc.vector.tensor_copy(vbc[:], pb[:, 0:HD])
        for ob in range(2):
            pb2 = ps.tile([128, 512], FP32, tag="sp")
            nc.tensor.matmul(pb2[:], lhsT=ones32[:],
                             rhs=bo_row[:, ob * 512:(ob + 1) * 512],
                             start=True, stop=True)
            nc.vector.tensor_copy(obc[:, ob, :], pb2[:])
        for _ in range(N_WARM):
            pw = ps.tile([128, 512], FP32, tag="sp")
            nc.tensor.matmul(pw[:], lhsT=wdum[:], rhs=rdum[:],
                             start=True, stop=True)

        # ---------------- projection chains ----------------
        def qk_chain(wT, b_sb, dstT, cb, t0, t1):
            pp = ps.tile([128, t1 - t0], FP32, tag="sp")
            for dc in range(DC):
                nc.tensor.matmul(
                    pp[:],
                    lhsT=wT[:, dc, cb * 128:(cb + 1) * 128],
                    rhs=xt[:, dc, t0:t1],
                    start=(dc == 0),
                    stop=(dc == DC - 1),
                )
            # evacuate + per-partition bias + cast to fp16 on DVE
            nc.vector.tensor_scalar(
                dstT[:, cb, t0:t1], pp[:], b_sb[:, cb, :], None, ALU.add,
            )

        def v_chain(jb):
            pv = ps.tile([128, 512], FP32, tag="sp")
            for dc in range(DC):
                nc.tensor.matmul(
                    pv[:, 0:HD],
                    lhsT=xt[:, dc, jb * 128:(jb + 1) * 128],
                    rhs=wvT[:, dc, :],
                    start=(dc == 0),
                    stop=(dc == DC - 1),
                )
            nc.vector.tensor_tensor(vh[:, jb, :, 0:64], pv[:, 0:HD], vbc[:], ALU.add)

        # q(ib=0) on the two half-blocks, then k/v in x-arrival order.
        for t0, t1 in ((0, 256), (256, 512)):
            qk_chain(wqT, bq_sb, qT, 0, t0, t1)
            qk_chain(wqT, bq_sb, qT, 1, t0, t1)
        qk_chain(wkT, bk_sb, kT, 0, 0, 512)
        qk_chain(wkT, bk_sb, kT, 1, 0, 512)
        for jb in range(4):
            v_chain(jb)
        for tb in range(1, 4):
            ts0, ts1 = tb * 512, (tb + 1) * 512
            qk_chain(wkT, bk_sb, kT, 0, ts0, ts1)
            qk_chain(wkT, bk_sb, kT, 1, ts0, ts1)
            for jb in range(tb * 4, (tb + 1) * 4):
                v_chain(jb)

        # ---------------- attention + out-projection ----------------
        def oproj_tile(it, ob):
            # out[i, do] = sum_c yT[c, i] * WoT[c, do] + bo[do]
            tsl = slice(it * 128, (it + 1) * 128)
            po = ps.tile([128, 512], FP32, tag="sp")
            osl = slice(ob * 512, (ob + 1) * 512)
            for cc2 in range(2):
                nc.tensor.matmul(
                    po[:],
                    lhsT=yT[:, cc2, tsl],
                    rhs=woT[:, cc2, osl],
                    start=(cc2 == 0), stop=(cc2 == 1),
                )
            osb = osb_pool.tile([128, 512], FP32)
            nc.vector.tensor_tensor(osb[:], po[:], obc[:, ob, :], ALU.add)
            nc.sync.dma_start(out_ap[tsl, osl], osb[:])

        def q_filler(ib2, cb):
            qk_chain(wqT, bq_sb, qT, cb, ib2 * 512, (ib2 + 1) * 512)

        # filler quanta per ib: list of closures issued at fixed jb slots
        fillers = {ib: [] for ib in range(NIB)}
        fillers[0] = [lambda: q_filler(1, 0), lambda: q_filler(1, 1),
                      lambda: q_filler(2, 0), lambda: q_filler(2, 1),
                      lambda: q_filler(3, 0), lambda: q_filler(3, 1)]
        for ib in range(1, NIB):
            prev = ib - 1
            fillers[ib] = [
                (lambda it=prev * 4 + s, ob=o: oproj_tile(it, ob))
                for s in range(4) for o in range(2)
            ]

        for ib in range(NIB):
            isl = slice(ib * 512, (ib + 1) * 512)
            fl = fillers[ib]
            fi = 0
            for cc in range(2):          # head pair (2*cc, 2*cc+1)
                av0 = ps.tile([65, 512], FP32, tag="av0", bufs=1)
                av1 = ps.tile([65, 512], FP32, tag="av1", bufs=1)
                sps = {}
                exs = {}

                def emit_S(jb):
                    jsl = slice(jb * 128, (jb + 1) * 128)
                    sp = ps.tile([128, 1024], FP32, tag="sp")
                    nc.tensor.matmul(
                        sp[:, 0:512],
                        lhsT=kT[0:64, cc, jsl], rhs=qT[0:64, cc, isl],
                        start=True, stop=True,
                    )
                    nc.tensor.matmul(
                        sp[:, 512:1024],
                        lhsT=kT[64:128, cc, jsl], rhs=qT[64:128, cc, isl],
                        start=True, stop=True,
                    )
                    ex = exps_pool.tile([128, 1024], BF16)
                    nc.scalar.activation(ex[:], sp[:], AF.Exp, bias=expb[:])
                    exs[jb] = ex

                def emit_AV(jb):
                    ex = exs.pop(jb)
                    first, last = (jb == 0), (jb == NJB - 1)
                    nc.tensor.matmul(
                        av0[:],
                        lhsT=vh[:, jb, 2 * cc, 0:65], rhs=ex[:, 0:512],
                        start=first, stop=last,
                    )
                    nc.tensor.matmul(
                        av1[:],
                        lhsT=vh[:, jb, 2 * cc + 1, 0:65], rhs=ex[:, 512:1024],
                        start=first, stop=last,
                    )

                emit_S(0)
                for jb in range(1, NJB):
                    emit_S(jb)
                    emit_AV(jb - 1)
                    if jb % 4 == 3 and fi < len(fl):
                        fl[fi]()
                        fi += 1
                emit_AV(NJB - 1)

                # ---- normalization: y^T_h = av_h * bcast(1/(16*denom)) ----
                # avs copies free the av PSUM banks for the next sweep.
                avs0 = norm_pool.tile([65, 512], FP32, tag="avs0")
                nc.vector.tensor_copy(avs0[:], av0[:])
                avs1 = norm_pool.tile([65, 512], FP32, tag="avs1")
                nc.vector.tensor_copy(avs1[:], av1[:])
                # r = 1/(16*denom) in fp32; r spans ~1e-23..1e0 so the 16-bit
                # broadcast operand must be bf16 (fp16 underflows); a hi/lo
                # split accumulated in PSUM keeps ~2^-16 relative accuracy.
                rec = norm_pool.tile([33, 512], FP32, tag="rec")
                nc.vector.reciprocal(rec[0:1, :], avs0[64:65, :])
                nc.vector.reciprocal(rec[32:33, :], avs1[64:65, :])
                rhs4 = norm_pool.tile([97, 512], BF16, tag="rhs4")
                nc.vector.tensor_copy(rhs4[0:33, :], rec[:])
                with nc.allow_low_precision(reason="lo part of bf16 hi/lo split"):
                    nc.vector.tensor_tensor(rhs4[64:97, :], rec[:], rhs4[0:33, :],
                                            ALU.subtract)
                bc = ps.tile([128, 512], FP32, tag="sp")
                nc.tensor.matmul(bc[:], lhsT=nlhsT[:], rhs=rhs4[:],
                                 start=True, stop=True)
                nc.vector.tensor_tensor(
                    yT[0:64, cc, isl], avs0[0:64, :], bc[0:64, :], ALU.mult,
                )
                nc.vector.tensor_tensor(
                    yT[64:128, cc, isl], avs1[0:64, :], bc[64:128, :], ALU.mult,
                )
            while fi < len(fl):
                fl[fi]()
                fi += 1
        # tail: out-projection of the last i-block
        for s in range(4):
            for o in range(2):
                oproj_tile(12 + s, o)


_CACHE: dict = {}


def _get_compiled():
    key = "nc"
    if key in _CACHE:
        return _CACHE[key]
    nc = bacc.Bacc(
        "TRN2", target_bir_lowering=False, debug=False, num_devices=NCORES,
    )
    x = nc.dram_tensor("x", (N, D), FP32, kind="ExternalInput").ap()
    wqk = nc.dram_tensor("Wqk", (2 * HD, D), FP32, kind="ExternalInput").ap()
    bqk = nc.dram_tensor("bqk", (2 * HD,), FP32, kind="ExternalInput").ap()
    wv = nc.dram_tensor("Wv", (HD, D), FP32, kind="ExternalInput").ap()
    bv = nc.dram_tensor("bv", (HD,), FP32, kind="ExternalInput").ap()
    wo = nc.dram_tensor("Wo", (D, HD), FP32, kind="ExternalInput").ap()
    bo = nc.dram_tensor("bo", (D,), FP32, kind="ExternalInput").ap()
    out = nc.dram_tensor("out", (N, D), FP32, kind="ExternalOutput").ap()

    with tile.TileContext(nc) as tc:
        _build_kernel(nc, tc, out, x, wqk, bqk, wv, bv, wo, bo)
    nc.compile()
    _CACHE[key] = nc
    return nc


def run_cores(in_maps, trace=False, **kw):
    nc = _get_compiled()
    return bass_utils.run_bass_kernel_spmd(
        nc, in_maps, core_ids=list(range(NCORES)), trace=trace, **kw
    )


def kernel(x, Wqk, bqk, Wv, bv, Wo, bo):
    x = np.asarray(x, dtype=np.float32)
    in_maps = [
        {
            "x": np.ascontiguousarray(x[c]),
            "Wqk": np.asarray(Wqk, np.float32),
            "bqk": np.asarray(bqk, np.float32),
            "Wv": np.asarray(Wv, np.float32),
            "bv": np.asarray(bv, np.float32),
            "Wo": np.asarray(Wo, np.float32),
            "bo": np.asarray(bo, np.float32),
        }
        for c in range(NCORES)
    ]
    # The axon tunnel occasionally returns a glitched execution (transient
    # non-finite garbage); retry a couple of times in that case.
    for _attempt in range(3):
        res = run_cores(in_maps)
        out = np.stack([res.results[c]["out"] for c in range(NCORES)], axis=0)
        if np.isfinite(out).all():
            break
    return out


# revision 16
# speedup vs baseline: 1.1008x; 1.1008x over previous
"""Trainium2 Bass kernel for the (misordered-scale) MultiHeadAttention problem.

Problem (per batch b of 8, one NeuronCore each):
  qk = x @ Wqk.T + bqk            # [2048, 512], channel c = 2*(h*64+e) + {0:q, 1:k}
  v  = x @ Wv.T  + bv             # [2048, 256], channel c = h*64+e
  S_h = q_h @ k_h.T               # [2048, 2048] per head (e=64)
  attn = softmax(S, -1) / 16
  out_h = attn_h @ v_h            # [2048, 64]
  out = concat_h(out_h) @ Wo.T + bo   # [2048, 1024]

Strategy: data-parallel over batch across 8 cores (no collectives).

Schedule (single dense PE stream, cost-model-guided):
  - x / Wq / Wk / Wv are consumed as float32r straight from HBM via 4-byte
    DMA-transposes (two 64-partition half-d calls each) -- no fp16 cast pass,
    so the first projection chain starts ~4us in.  fp32r streams at the same
    1 cycle/row as fp16 for >=256-wide outputs.
  - Projections run as 8-matmul accumulation chains double-buffered across
    two PSUM tags; q(ib=0) + all k + all v run up front (in x-arrival
    order), q(ib=1..3) chains are issued inside the attention phase as PE
    filler.
  - Attention is software-pipelined per (ib, head-pair): S(jb+1) is issued
    between exp(jb) (ACT, fused bias -8) and AV(jb); out-projection tiles and
    q-chains fill the PE gap left by the slower ACT exp stream.
  - The AV stationary operand [v_h | 16.0] makes PSUM row 64 the
    softmax denominator times 16 (folding the 1/16 mis-scale for free).
  - Normalization: one K=4 rank-4 PE matmul per (ib, pair) broadcasts both
    heads' 1/(16*denom) (bf16 hi/lo split for ~2^-16 accuracy) along
    partitions.
  - Wo stays fp16 (cast + transpose DMA); out tiles are evacuated with the
    bo broadcast on DVE and stored straight from SBUF.
"""

import numpy as np
from contextlib import ExitStack

import concourse.bass as bass
import concourse.mybir as mybir
import concourse.tile as tile
from concourse import bacc
from concourse import bass_utils

FP32 = mybir.dt.float32
FP32R = mybir.dt.float32r
BF16 = mybir.dt.bfloat16
FP16 = mybir.dt.float16
AF = mybir.ActivationFunctionType
ALU = mybir.AluOpType

B = 8
N = 2048          # tokens per batch
D = 1024          # model dim
H = 4             # heads
E = 64            # per-head dim after the einops split
HD = 256          # H*E (v channels / Wo contraction dim)
NCORES = 8

DC = D // 128      # 8 d-chunks of 128
NIB = N // 512     # 4 i-blocks of 512
NJB = N // 128     # 16 j-blocks of 128
# exp(S - 8): headroom offset for the exp path (max logit ~51 -> e^43 fits
# bf16 comfortably); the offset cancels exactly in the softmax normalization.
EXP_BIAS = -8.0
# AV ones-column value: makes PSUM row 64 equal 16*sum(exp), so its
# reciprocal is directly the softmax/16 normalization factor.
DEN_SCALE = 16.0

N_WARM = 14        # PE warmup matmuls (keep PE busy until first chain is fed)


def _build_kernel(nc: bass.Bass, tc: tile.TileContext, out_ap, x, wqk, bqk, wv, bv, wo, bo):
    ctx = ExitStack()
    with ctx:
        consts = ctx.enter_context(tc.tile_pool(name="consts", bufs=1))
        dram = ctx.enter_context(tc.tile_pool(name="dram", bufs=1, space="DRAM"))
        exps_pool = ctx.enter_context(tc.tile_pool(name="exps", bufs=4))
        osb_pool = ctx.enter_context(tc.tile_pool(name="osb", bufs=4))
        norm_pool = ctx.enter_context(tc.tile_pool(name="norm", bufs=2))
        ps = ctx.enter_context(tc.tile_pool(name="ps", bufs=3, space="PSUM"))

        # ---------------- persistent SBUF tensors ----------------
        xt = consts.tile([128, DC, N], FP16)          # x^T: xt[p, dc, t] = x[t, p*8+dc]
        wqT = consts.tile([128, DC, HD], FP16)        # Wq^T: [d, c]
        wkT = consts.tile([128, DC, HD], FP16)
        wvT = consts.tile([128, DC, HD], FP16)
        woT = consts.tile([128, 2, D], FP16)          # Wo^T: [c, do]
        qT = consts.tile([128, 2, N], FP16)           # q^T: [c, i]; c = cc*128 + p
        kT = consts.tile([128, 2, N], FP16)
        yT = consts.tile([128, 2, N], FP16)           # concat-head attn out, feature-major
        vh = consts.tile([128, NJB, H, 66], BF16)     # [j, jb, h, 0:64]=v_h, [...,64]=16.0
        bq_sb = consts.tile([128, 2, 1], FP32)        # q bias per partition (c)
        bk_sb = consts.tile([128, 2, 1], FP32)
        bv_row = consts.tile([1, HD], FP32)           # v bias as K=1 matmul rhs
        bo_row = consts.tile([1, D], FP32)
        ones32 = consts.tile([1, 128], FP32)
        wdum = consts.tile([1, 128], FP16)            # warmup stationary
        rdum = consts.tile([1, 512], FP16)            # warmup moving
        # Rank-4 norm broadcast lhsT.  Engine writes must start at a partition
        # quadrant (0/32/64/96), so the 4 logical rows (rhi0, rhi1, rlo0,
        # rlo1) live at partitions 0/32/64/96 of a 97-partition tile whose
        # other rows are hard zeros; the matmul contracts over all 97 rows at
        # the same cost (PE time only depends on the 512-wide output).
        nlhsT = consts.tile([97, 128], BF16)
        expb = consts.tile([128, 1], FP32)
        vbc = consts.tile([128, HD], FP32)            # bv broadcast over tokens
        obc = consts.tile([128, 2, 512], FP32)        # bo broadcast over tokens
        nc.vector.memset(ones32[:], 1.0)
        nc.vector.memset(wdum[:], 0.0)
        nc.vector.memset(rdum[:], 0.0)
        nc.vector.memset(expb[:], EXP_BIAS)
        nc.vector.memset(vh[:, :, :, 64:66], DEN_SCALE)
        nc.vector.memset(nlhsT[:], 0.0)
        nc.vector.memset(nlhsT[0:1, 0:64], 1.0)       # rhi h0 -> out rows 0:64
        nc.vector.memset(nlhsT[32:33, 64:128], 1.0)   # rhi h1 -> out rows 64:128
        nc.vector.memset(nlhsT[64:65, 0:64], 1.0)     # rlo h0
        nc.vector.memset(nlhsT[96:97, 64:128], 1.0)   # rlo h1
        # The rec / rhs4 norm tiles are written only at partition-quadrant
        # rows; zero both rotating buffers once so the filler rows contract
        # as exact zeros (never NaN garbage).
        for _ in range(2):
            rz = norm_pool.tile([33, 512], FP32, tag="rec")
            nc.vector.memset(rz[:], 0.0)
            hz = norm_pool.tile([97, 512], BF16, tag="rhs4")
            nc.vector.memset(hz[:], 0.0)

        # ---------------- DRAM staging (fp16 casts) ----------------
        # One DRAM tile per transfer: the tile scheduler tracks DRAM tiles at
        # whole-tile granularity, so a shared staging buffer would serialize
        # every cast against every transpose.  Three rotating 256-token x
        # stages throttle the cast stream via real WAR dependencies -- cast i
        # must wait for transpose i-3 -- which forces the scheduler to
        # interleave transposes with casts on the shared DMA device instead
        # of running every cast first.
        xstg = [dram.tile([256, D], FP16, name=f"xst{i}") for i in range(3)]
        wq_bf = dram.tile([HD, D], FP16)
        wk_bf = dram.tile([HD, D], FP16)
        wv_bf = dram.tile([HD, D], FP16)
        wo_bf = dram.tile([D, HD], FP16)

        # De-interleave Wqk rows: q rows are 2c, k rows are 2c+1.
        wqk_r = wqk.rearrange("(c s) d -> s c d", s=2)
        bqk_r = bqk.rearrange("(c s) -> s c", s=2)

        # bias loads ride the ACT HWDGE ring (cheap, off the SP transpose ring)
        nc.scalar.dma_start(bv_row[0:1, :], bv[:])
        nc.scalar.dma_start(bo_row[0:1, :], bo[:])
        for cb in range(2):
            nc.scalar.dma_start(bq_sb[:, cb, :], bqk_r[0, cb * 128:(cb + 1) * 128])
            nc.scalar.dma_start(bk_sb[:, cb, :], bqk_r[1, cb * 128:(cb + 1) * 128])

        # ---------------- casts + transposes, supply order ----------------
        # The d (contraction) axis uses a permuted internal layout: SBUF chunk
        # dc at partition p holds global d = p*8 + dc (consistent between xt
        # and all W^T tiles; d is purely internal), so one full-width
        # transpose-DMA fills all 8 chunks via a 3D output AP.  Casts ride the
        # Pool SWDGE ring; x transposes the SP HWDGE ring and weight
        # transposes the ACT HWDGE ring (two rings keep two transfers in
        # flight while the casts flood the shared DMA device); the device
        # itself serves transfers roughly in arrival order, so the emission
        # order below is the supply schedule: x block 0 (split for an early
        # start) and the q/k weights first, wv before x block 1, then Wo.
        def x_sup(ci):
            t0, t1 = ci * 256, (ci + 1) * 256
            st = xstg[ci % 3]
            nc.gpsimd.dma_start(st[:, :], x[t0:t1, :])
            nc.sync.dma_start(xt[:, :, t0:t1], st[:, :], transpose=True)

        def w_sup(w_stage, dstT, src):
            nc.gpsimd.dma_start(w_stage[:], src)
            nc.sync.dma_start(dstT[:], w_stage[:], transpose=True)

        x_sup(0)
        w_sup(wq_bf, wqT, wqk_r[0])
        x_sup(1)
        w_sup(wk_bf, wkT, wqk_r[1])
        x_sup(2)
        x_sup(3)
        w_sup(wv_bf, wvT, wv[:])
        for ci in range(4, 8):
            x_sup(ci)
        nc.gpsimd.dma_start(wo_bf[:], wo[:])
        for g in range(2):
            # Wo^T must match yT's c-layout (c = cc*128 + p): per-chunk.
            cs = slice(g * 128, (g + 1) * 128)
            nc.sync.dma_start(woT[:, g, :], wo_bf[:, cs], transpose=True)

        # ---------------- PE warmup + bias broadcasts ----------------
        # Keep the PE stream dense from ~1us so the p-state ramp is spent on
        # throwaway work, and the first real chains run at full clock.
        pb = ps.tile([128, 512], FP32, tag="sp")
        nc.tensor.matmul(pb[:, 0:HD], lhsT=ones32[:], rhs=bv_row[:],
                         start=True, stop=True)
        nc.vector.tensor_copy(vbc[:], pb[:, 0:HD])
        for ob in range(2):
            pb2 = ps.tile([128, 512], FP32, tag="sp")
            nc.tensor.matmul(pb2[:], lhsT=ones32[:],
                             rhs=bo_row[:, ob * 512:(ob + 1) * 512],
                             start=True, stop=True)
            nc.vector.tensor_copy(obc[:, ob, :], pb2[:])
        for _ in range(N_WARM):
            pw = ps.tile([128, 512], FP32, tag="sp")
            nc.tensor.matmul(pw[:], lhsT=wdum[:], rhs=rdum[:],
                             start=True, stop=True)

        # ---------------- projection chains ----------------
        def qk_chain(wT, b_sb, dstT, cb, t0, t1):
            pp = ps.tile([128, t1 - t0], FP32, tag="sp")
            for dc in range(DC):
                nc.tensor.matmul(
                    pp[:],
                    lhsT=wT[:, dc, cb * 128:(cb + 1) * 128],
                    rhs=xt[:, dc, t0:t1],
                    start=(dc == 0),
                    stop=(dc == DC - 1),
                )
            # evacuate + per-partition bias + cast to fp16 on DVE
            nc.vector.tensor_scalar(
                dstT[:, cb, t0:t1], pp[:], b_sb[:, cb, :], None, ALU.add,
            )

        def v_chain(jb):
            pv = ps.tile([128, 512], FP32, tag="sp")
            for dc in range(DC):
                nc.tensor.matmul(
                    pv[:, 0:HD],
                    lhsT=xt[:, dc, jb * 128:(jb + 1) * 128],
                    rhs=wvT[:, dc, :],
                    start=(dc == 0),
                    stop=(dc == DC - 1),
                )
            nc.vector.tensor_tensor(vh[:, jb, :, 0:64], pv[:, 0:HD], vbc[:], ALU.add)

        # q(ib=0) on the two half-blocks, then k/v in x-arrival order.
        for t0, t1 in ((0, 256), (256, 512)):
            qk_chain(wqT, bq_sb, qT, 0, t0, t1)
            qk_chain(wqT, bq_sb, qT, 1, t0, t1)
        qk_chain(wkT, bk_sb, kT, 0, 0, 512)
        qk_chain(wkT, bk_sb, kT, 1, 0, 512)
        for jb in range(4):
            v_chain(jb)
        for tb in range(1, 4):
            ts0, ts1 = tb * 512, (tb + 1) * 512
            qk_chain(wkT, bk_sb, kT, 0, ts0, ts1)
            qk_chain(wkT, bk_sb, kT, 1, ts0, ts1)
            for jb in range(tb * 4, (tb + 1) * 4):
                v_chain(jb)

        # ---------------- attention + out-projection ----------------
        def oproj_tile(it, ob):
            # out[i, do] = sum_c yT[c, i] * WoT[c, do] + bo[do]
            tsl = slice(it * 128, (it + 1) * 128)
            po = ps.tile([128, 512], FP32, tag="sp")
            osl = slice(ob * 512, (ob + 1) * 512)
            for cc2 in range(2):
                nc.tensor.matmul(
                    po[:],
                    lhsT=yT[:, cc2, tsl],
                    rhs=woT[:, cc2, osl],
                    start=(cc2 == 0), stop=(cc2 == 1),
                )
            osb = osb_pool.tile([128, 512], FP32)
            nc.vector.tensor_tensor(osb[:], po[:], obc[:, ob, :], ALU.add)
            nc.sync.dma_start(out_ap[tsl, osl], osb[:])

        # q-chains are split into two ~4-matmul filler quanta so they slot
        # between S pairs without draining the exp stream.  Parts A and B of
        # one chain must land on consecutive filler slots: with 3 sp slabs
        # the chain's accumulator survives exactly two interleaved S-pair
        # allocations.
        def q_fill_pair(ib2, cb):
            t0, t1 = ib2 * 512, (ib2 + 1) * 512
            state = {}

            def part_a():
                pp = ps.tile([128, 512], FP32, tag="sp", name="qf")
                state["pp"] = pp
                for dc in range(4):
                    nc.tensor.matmul(
                        pp[:],
                        lhsT=wqT[:, dc, cb * 128:(cb + 1) * 128],
                        rhs=xt[:, dc, t0:t1],
                        start=(dc == 0), stop=False,
                    )

            def part_b():
                pp = state["pp"]
                for dc in range(4, DC):
                    nc.tensor.matmul(
                        pp[:],
                        lhsT=wqT[:, dc, cb * 128:(cb + 1) * 128],
                        rhs=xt[:, dc, t0:t1],
                        start=False, stop=(dc == DC - 1),
                    )
                nc.vector.tensor_scalar(
                    qT[:, cb, t0:t1], pp[:], bq_sb[:, cb, :], None, ALU.add,
                )
            return [part_a, part_b]

        # Per-sweep filler queues (sweep s = ib*2 + cc).  The deferred norm
        # tail of sweep s-1 is prepended inside the loop.
        sweep_fillers = {s: [] for s in range(2 * NIB + 1)}
        sweep_fillers[0] = q_fill_pair(1, 0) + q_fill_pair(1, 1) \
            + q_fill_pair(2, 0)
        sweep_fillers[1] = q_fill_pair(2, 1) + q_fill_pair(3, 0) \
            + q_fill_pair(3, 1)
        for ib in range(1, NIB):
            prev = ib - 1
            tiles = [(prev * 4 + t, o) for t in range(4) for o in range(2)]
            for k in range(2):
                sweep_fillers[2 * ib + k] += [
                    (lambda it=it, ob=ob: oproj_tile(it, ob))
                    for it, ob in tiles[k * 4:(k + 1) * 4]
                ]
        sweep_fillers[2 * NIB] = [
            (lambda it=12 + t, ob=o: oproj_tile(it, ob))
            for t in range(4) for o in range(2)
        ]

        for ib in range(NIB):
            isl = slice(ib * 512, (ib + 1) * 512)
            for cc in range(2):          # head pair (2*cc, 2*cc+1)
                s_idx = ib * 2 + cc
                fl = sweep_fillers[s_idx]
                fi = 0
                av0 = ps.tile([65, 512], FP32, tag="av0", bufs=1)
                av1 = ps.tile([65, 512], FP32, tag="av1", bufs=1)
                exs = {}

                def emit_S(jb):
                    jsl = slice(jb * 128, (jb + 1) * 128)
                    sp = ps.tile([128, 1024], FP32, tag="sp")
                    nc.tensor.matmul(
                        sp[:, 0:512],
                        lhsT=kT[0:64, cc, jsl], rhs=qT[0:64, cc, isl],
                        start=True, stop=True,
                    )
                    nc.tensor.matmul(
                        sp[:, 512:1024],
                        lhsT=kT[64:128, cc, jsl], rhs=qT[64:128, cc, isl],
                        start=True, stop=True,
                    )
                    ex = exps_pool.tile([128, 1024], BF16)
                    nc.scalar.activation(ex[:], sp[:], AF.Exp, bias=expb[:])
                    exs[jb] = ex

                def emit_AV(jb):
                    ex = exs.pop(jb)
                    first, last = (jb == 0), (jb == NJB - 1)
                    nc.tensor.matmul(
                        av0[:],
                        lhsT=vh[:, jb, 2 * cc, 0:65], rhs=ex[:, 0:512],
                        start=first, stop=last,
                    )
                    nc.tensor.matmul(
                        av1[:],
                        lhsT=vh[:, jb, 2 * cc + 1, 0:65], rhs=ex[:, 512:1024],
                        start=first, stop=last,
                    )

                emit_S(0)
                for jb in range(1, NJB):
                    emit_S(jb)
                    emit_AV(jb - 1)
                    if jb % 2 == 1 and fi < len(fl):
                        fl[fi]()
                        fi += 1
                emit_AV(NJB - 1)
                while fi < len(fl):
                    fl[fi]()
                    fi += 1

                # ---- normalization: y^T_h = av_h * bcast(1/(16*denom)) ----
                # The DVE chain runs now (avs copies free the av banks for
                # the next sweep); the PE broadcast matmul + yT multiplies
                # are deferred into the next sweep's filler queue so the PE
                # stream never waits on this chain.
                avs0 = norm_pool.tile([65, 512], FP32, tag="avs0")
                nc.vector.tensor_copy(avs0[:], av0[:])
                avs1 = norm_pool.tile([65, 512], FP32, tag="avs1")
                nc.vector.tensor_copy(avs1[:], av1[:])
                # r = 1/(16*denom) in fp32; r spans ~1e-23..1e0 so the 16-bit
                # broadcast operand must be bf16 (fp16 underflows); a hi/lo
                # split accumulated in PSUM keeps ~2^-16 relative accuracy.
                rec = norm_pool.tile([33, 512], FP32, tag="rec")
                nc.vector.reciprocal(rec[0:1, :], avs0[64:65, :])
                nc.vector.reciprocal(rec[32:33, :], avs1[64:65, :])
                rhs4 = norm_pool.tile([97, 512], BF16, tag="rhs4")
                nc.vector.tensor_copy(rhs4[0:33, :], rec[:])
                with nc.allow_low_precision(reason="lo part of bf16 hi/lo split"):
                    nc.vector.tensor_tensor(rhs4[64:97, :], rec[:], rhs4[0:33, :],
                                            ALU.subtract)

                def norm_tail(cc=cc, isl=isl, avs0=avs0, avs1=avs1, rhs4=rhs4):
                    bc = ps.tile([128, 512], FP32, tag="sp")
                    nc.tensor.matmul(bc[:], lhsT=nlhsT[:], rhs=rhs4[:],
                                     start=True, stop=True)
                    nc.vector.tensor_tensor(
                        yT[0:64, cc, isl], avs0[0:64, :], bc[0:64, :], ALU.mult,
                    )
                    nc.vector.tensor_tensor(
                        yT[64:128, cc, isl], avs1[0:64, :], bc[64:128, :],
                        ALU.mult,
                    )
                sweep_fillers[s_idx + 1].insert(0, norm_tail)
        # tail: last norm broadcast + out-projection of the last i-block
        for f in sweep_fillers[2 * NIB]:
            f()


_CACHE: dict = {}


def _get_compiled():
    key = "nc"
    if key in _CACHE:
        return _CACHE[key]
    nc = bacc.Bacc(
        "TRN2", target_bir_lowering=False, debug=False, num_devices=NCORES,
    )
    x = nc.dram_tensor("x", (N, D), FP32, kind="ExternalInput").ap()
    wqk = nc.dram_tensor("Wqk", (2 * HD, D), FP32, kind="ExternalInput").ap()
    bqk = nc.dram_tensor("bqk", (2 * HD,), FP32, kind="ExternalInput").ap()
    wv = nc.dram_tensor("Wv", (HD, D), FP32, kind="ExternalInput").ap()
    bv = nc.dram_tensor("bv", (HD,), FP32, kind="ExternalInput").ap()
    wo = nc.dram_tensor("Wo", (D, HD), FP32, kind="ExternalInput").ap()
    bo = nc.dram_tensor("bo", (D,), FP32, kind="ExternalInput").ap()
    out = nc.dram_tensor("out", (N, D), FP32, kind="ExternalOutput").ap()

    with tile.TileContext(nc) as tc:
        _build_kernel(nc, tc, out, x, wqk, bqk, wv, bv, wo, bo)
    nc.compile()
    _CACHE[key] = nc
    return nc


def run_cores(in_maps, trace=False, **kw):
    nc = _get_compiled()
    return bass_utils.run_bass_kernel_spmd(
        nc, in_maps, core_ids=list(range(NCORES)), trace=trace, **kw
    )


def kernel(x, Wqk, bqk, Wv, bv, Wo, bo):
    x = np.asarray(x, dtype=np.float32)
    in_maps = [
        {
            "x": np.ascontiguousarray(x[c]),
            "Wqk": np.asarray(Wqk, np.float32),
            "bqk": np.asarray(bqk, np.float32),
            "Wv": np.asarray(Wv, np.float32),
            "bv": np.asarray(bv, np.float32),
            "Wo": np.asarray(Wo, np.float32),
            "bo": np.asarray(bo, np.float32),
        }
        for c in range(NCORES)
    ]
    # The axon tunnel occasionally returns a glitched execution (transient
    # non-finite garbage); retry a couple of times in that case.
    for _attempt in range(3):
        res = run_cores(in_maps)
        out = np.stack([res.results[c]["out"] for c in range(NCORES)], axis=0)
        if np.isfinite(out).all():
            break
    return out
